# revision 23
# baseline (speedup 1.0000x reference)
"""Trainium2 Bass kernel for the BPIC MIMO detector (nn_BPICDetector).

65536 independent (batch x subcarrier) problems: per problem a 16x8 complex
channel H, 16-vector y, `internal_it` BPIC iterations of BSO (MRC parallel
interference cancellation) + BSE (Bayesian symbol estimation over 16-QAM),
producing x_bse, v_bse, ise_dsc (8 complex values each).

Layout: 128 problems on SBUF partitions per tile; the small per-problem
algebra lives on the free axis using stride-0 broadcast APs and grouped
free-dim reductions. fp32 throughout (exp(w*d2) amplifies absolute error by
|w| ~ 2e2, ruling out 16-bit intermediates).

Validated restructurings (proto.py: lands at the fp32 reimplementation noise
floor ~8e-3 scale-relative absmax vs the jax reference):
  - no whitening: No cancels except v_bso's No^2/diag term
  - prior==0 => initial moments are compile-time constants x0, v0
  - constellation as an (rc, ic) 4x4 grid; dist^2 separable over PAM axes;
    stabilizer = signed extremum of d2, real part only (imag shift cancels
    in the pdf normalization)
  - v_bse via expansion E/S - 2*(xr*U + xi*V)/S + |x|^2
  - Off@x_bse matvec shared between ise_dsc and the next iteration's BSO
"""

import math
import numpy as np

import concourse.bass as bass
import concourse.mybir as mybir
import concourse.tile as tile
from concourse import bacc, bass_utils

F32 = mybir.dt.float32
AX = mybir.AxisListType
OP = mybir.AluOpType
ACT = mybir.ActivationFunctionType

B, S, M, K, C = 128, 512, 16, 8, 16
NCORES = 8
PT = 128                      # problems per tile (partition dim)
NP = B * S // NCORES          # problems per core = 8192
UNROLL = 2                    # ptiles per loop iteration

# ---- constellation constants (host side, replicating reference order) ----
_pam_raw = np.array([-3.0, -1.0, 3.0, 1.0], dtype=np.float32)
_pts = []
for _i in range(C):
    _b = [(_i >> (3 - _k)) & 1 for _k in range(4)]
    _pts.append(_pam_raw[(_b[0] << 1) | _b[2]] + 1j * _pam_raw[(_b[1] << 1) | _b[3]])
_POINTS = np.asarray(_pts, dtype=np.complex64)
_POINTS = _POINTS / np.float32(np.sqrt(np.mean(np.abs(_POINTS) ** 2)))
PAMV = (_pam_raw / np.float32(np.sqrt(10.0))).astype(np.float32)

_p0 = np.full((C,), 1.0 / 16.0, dtype=np.float32)
X0 = np.sum(_p0.astype(np.complex64) * _POINTS)          # ~0 complex
V0 = np.float32(np.sum(_p0 * np.abs(_POINTS) ** 2) - np.abs(X0) ** 2)

# consts tensor columns
C_PAM = 0     # 0..3   pam values
C_PSQ = 4     # 4..7   pam^2
C_X0R = 8
C_X0I = 9
C_NX0I = 10   # -x0i
C_V0 = 11
C_C0 = 12     # No^2 (runtime)
C_HPI = 13    # pi/2
C_NPI = 14    # -pi
CW = 16


def _emit_ptile(nc, pools, hin_d, yin_d, out_d, pconst, row, T):
    """Emit the pipeline for one 128-problem tile; `row` = starting problem
    index (int or register expression)."""
    ts = nc.vector.tensor_scalar
    tt = nc.vector.tensor_tensor
    stt = nc.vector.scalar_tensor_tensor
    red = nc.vector.tensor_reduce
    act = nc.scalar.activation

    pam_c = pconst[:, C_PAM:C_PAM + 4]
    psq_c = pconst[:, C_PSQ:C_PSQ + 4]

    def sc(col):
        return pconst[:, col:col + 1]

    pool_in, pool_big, pool_mid, pool_small, pool_out = pools
    MK = M * K

    # ---- DMA in ----
    hraw = pool_in.tile([PT, 2 * MK], F32, tag="hraw")
    yraw = pool_in.tile([PT, 2 * M], F32, tag="yraw")
    nc.gpsimd.dma_start(hraw[:], hin_d[bass.ds(row, PT), :])
    nc.gpsimd.dma_start(yraw[:], yin_d[bass.ds(row, PT), :])

    # ---- S1 deinterleave (on ScalarE — DVE is the bottleneck engine) ----
    hh = pool_mid.tile([PT, 2 * MK], F32, tag="hh")     # hr | hi, each (m,k)
    yy = pool_small.tile([PT, 2 * M], F32, tag="yy")    # yr | yi
    nc.scalar.copy(hh[:, 0:MK], hraw[:, 0:2 * MK:2])
    nc.scalar.copy(hh[:, MK:2 * MK], hraw[:, 1:2 * MK:2])
    nc.scalar.copy(yy[:, 0:M], yraw[:, 0:2 * M:2])
    nc.scalar.copy(yy[:, M:2 * M], yraw[:, 1:2 * M:2])
    hr = hh[:, 0:MK]
    hi = hh[:, MK:2 * MK]
    yr = yy[:, 0:M]
    yi = yy[:, M:2 * M]

    # ---- S2 diag, invd (XY-reduce over (ri, m) at once) ----
    sq = pool_big.tile([PT, 2 * MK], F32, tag="sqh")
    act(sq[:, 0:MK], hr, ACT.Square)
    act(sq[:, MK:2 * MK], hi, ACT.Square)
    dtile = pool_small.tile([PT, 3 * K], F32, tag="dtile")  # diag | invd | invd2
    red(dtile[:, 0:K], sq[:].rearrange("p (r m k) -> p k r m", r=2, k=K), AX.XY, OP.add)
    nc.vector.reciprocal(dtile[:, K:2 * K], dtile[:, 0:K])
    invd = dtile[:, K:2 * K]
    tt(dtile[:, 2 * K:3 * K], invd, invd, OP.mult)
    invd2 = dtile[:, 2 * K:3 * K]

    # ---- S3 HtH (full 8x8 grid, k1-major, m innermost) ----
    def hA(x):  # [p, k1, k2, m], k2 broadcast
        return x.rearrange("p (m k) -> p k m", k=K).unsqueeze(2).to_broadcast((PT, K, K, M))

    def hB(x):  # [p, k1, k2, m], k1 broadcast
        return x.rearrange("p (m k) -> p k m", k=K).unsqueeze(1).to_broadcast((PT, K, K, M))

    def v4(t):
        return t[:].rearrange("p (a b m) -> p a b m", a=K, b=K)

    pr1 = pool_big.tile([PT, 2 * K * K * M], F32, tag="pr1")  # two product planes
    HH = pool_mid.tile([PT, 2 * K * K], F32, tag="HH")      # Or | Oi (k1,k2)
    KKM = K * K * M
    pl1 = pr1[:, 0:KKM].rearrange("p (a b m) -> p a b m", a=K, b=K)
    pl2 = pr1[:, KKM:2 * KKM].rearrange("p (a b m) -> p a b m", a=K, b=K)
    # XY-reduce over (plane, m) sums both product planes in one pass
    xy = pr1[:].rearrange("p (q a m) -> p a q m", q=2, m=M)
    hn = pool_mid.tile([PT, MK], F32, tag="hn")             # -hi
    ts(hn[:], hi, -1.0, None, OP.mult)
    tt(pl1, hA(hr), hB(hr), OP.mult)
    tt(pl2, hA(hi), hB(hi), OP.mult)
    red(HH[:, 0:K * K], xy, AX.XY, OP.add)
    tt(pl1, hA(hr), hB(hi), OP.mult)
    tt(pl2, hA(hn[:, 0:MK]), hB(hr), OP.mult)
    red(HH[:, K * K:2 * K * K], xy, AX.XY, OP.add)
    Or_ = HH[:, 0:K * K]
    Oi_ = HH[:, K * K:2 * K * K]

    # ---- S4 zero diag, Hsq, row sums ----
    nc.vector.memset(HH[:, 0:K * K:K + 1], 0.0)
    nc.vector.memset(HH[:, K * K:2 * K * K:K + 1], 0.0)
    QQ = pool_mid.tile([PT, 2 * K * K], F32, tag="QQ")      # Qr | Qi
    sqo = pool_mid.tile([PT, 2 * K * K], F32, tag="sqo")
    act(sqo[:, 0:K * K], Or_, ACT.Square)
    act(sqo[:, K * K:2 * K * K], Oi_, ACT.Square)
    tt(QQ[:, 0:K * K], sqo[:, 0:K * K], sqo[:, K * K:2 * K * K], OP.subtract)
    stt(QQ[:, K * K:2 * K * K], Or_, 2.0, Oi_, OP.mult, OP.mult)

    def g3(a):
        return a.rearrange("p (x y) -> p x y", y=K)

    rs = pool_small.tile([PT, 4 * K], F32, tag="rs")        # rs_or|rs_oi|rs_qr|rs_qi
    red(rs[:, 0:K], g3(Or_), AX.X, OP.add)
    red(rs[:, K:2 * K], g3(Oi_), AX.X, OP.add)
    red(rs[:, 2 * K:3 * K], g3(QQ[:, 0:K * K]), AX.X, OP.add)
    red(rs[:, 3 * K:4 * K], g3(QQ[:, K * K:2 * K * K]), AX.X, OP.add)

    # ---- S5 Hty ----
    def hA2(x):
        return x.rearrange("p (m k) -> p k m", k=K)

    def yB(x):
        return x.unsqueeze(1).to_broadcast((PT, K, M))

    def v3(t):
        return t[:].rearrange("p (k m) -> p k m", k=K)

    pm = pool_mid.tile([PT, 2 * K * M], F32, tag="pm")      # two product planes
    bt = pool_small.tile([PT, 2 * K], F32, tag="bt")        # btr | bti
    pma = pm[:, 0:K * M].rearrange("p (k m) -> p k m", k=K)
    pmb = pm[:, K * M:2 * K * M].rearrange("p (k m) -> p k m", k=K)
    pxy = pm[:].rearrange("p (q k m) -> p k q m", q=2, m=M)
    tt(pma, hA2(hr), yB(yr), OP.mult)
    tt(pmb, hA2(hi), yB(yi), OP.mult)
    red(bt[:, 0:K], pxy, AX.XY, OP.add)
    tt(pma, hA2(hr), yB(yi), OP.mult)
    stt(pmb, hA2(hi), -1.0, yB(yr), OP.mult, OP.mult)
    red(bt[:, K:2 * K], pxy, AX.XY, OP.add)

    # ---- iterations ----
    invd_b = invd.unsqueeze(1).to_broadcast((PT, 2, K))
    invd2_b = invd2.unsqueeze(1).to_broadcast((PT, 2, K))

    def ri(a):  # packed complex [p, 2K] -> [p, 2, K]
        return a.rearrange("p (r k) -> p r k", r=2)

    u_t = None
    vb_prev = None

    for itr in range(T):
        sv = pool_small.tile([PT, 2 * K], F32, tag="sv")
        if itr == 0:
            sx = pool_small.tile([PT, 2 * K], F32, tag="sx")
            t1 = pool_small.tile([PT, 2 * K], F32, tag="t1")
            ts(t1[:, 0:K], rs[:, 0:K], sc(C_X0R), None, OP.mult)
            stt(sx[:, 0:K], rs[:, K:2 * K], sc(C_NX0I), t1[:, 0:K], OP.mult, OP.add)
            ts(t1[:, K:2 * K], rs[:, K:2 * K], sc(C_X0R), None, OP.mult)
            stt(sx[:, K:2 * K], rs[:, 0:K], sc(C_X0I), t1[:, K:2 * K], OP.mult, OP.add)
            u_t = pool_small.tile([PT, 2 * K], F32, tag="u0")
            tt(u_t[:], bt[:], sx[:], OP.subtract)
            ts(sv[:], rs[:, 2 * K:4 * K], sc(C_V0), None, OP.mult)
        else:
            vrB = vb_prev[:, 0:K].unsqueeze(1).to_broadcast((PT, K, K))
            viB = vb_prev[:, K:2 * K].unsqueeze(1).to_broadcast((PT, K, K))
            Qr3 = g3(QQ[:, 0:K * K])
            Qi3 = g3(QQ[:, K * K:2 * K * K])
            mv1 = pool_mid.tile([PT, 2 * K * K], F32, tag="mv1")
            mva = mv1[:, 0:K * K].rearrange("p (a b) -> p a b", a=K)
            mvb = mv1[:, K * K:2 * K * K].rearrange("p (a b) -> p a b", a=K)
            mxy = mv1[:].rearrange("p (q a b) -> p a q b", q=2, b=K)
            tt(mva, Qr3, vrB, OP.mult)
            stt(mvb, Qi3, -1.0, viB, OP.mult, OP.mult)
            red(sv[:, 0:K], mxy, AX.XY, OP.add)
            tt(mva, Qr3, viB, OP.mult)
            tt(mvb, Qi3, vrB, OP.mult)
            red(sv[:, K:2 * K], mxy, AX.XY, OP.add)

        # x_bso = u * invd ; v_bso = c0*invd + s_v*invd2
        xbso = pool_small.tile([PT, 2 * K], F32, tag="xbso")
        tt(ri(xbso[:]), ri(u_t[:]), invd_b, OP.mult)
        vb_t = pool_small.tile([PT, 2 * K], F32, tag="vbso")
        tt(ri(vb_t[:]), ri(sv[:]), invd2_b, OP.mult)
        stt(vb_t[:, 0:K], invd, sc(C_C0), vb_t[:, 0:K], OP.mult, OP.add)

        # w = -0.5*conj(v_bso)/|v_bso|^2
        wt = pool_small.tile([PT, 2 * K], F32, tag="wt")
        r2 = pool_small.tile([PT, 2 * K], F32, tag="r2")
        tt(r2[:, 0:K], vb_t[:, 0:K], vb_t[:, 0:K], OP.mult)
        tt(r2[:, K:2 * K], vb_t[:, K:2 * K], vb_t[:, K:2 * K], OP.mult)
        tt(r2[:, 0:K], r2[:, 0:K], r2[:, K:2 * K], OP.add)
        nc.vector.reciprocal(r2[:, K:2 * K], r2[:, 0:K])
        stt(wt[:, 0:K], vb_t[:, 0:K], -0.5, r2[:, K:2 * K], OP.mult, OP.mult)
        stt(wt[:, K:2 * K], vb_t[:, K:2 * K], 0.5, r2[:, K:2 * K], OP.mult, OP.mult)
        wr = wt[:, 0:K]
        wi = wt[:, K:2 * K]

        # q = (x_bso - pam)^2 over (axis, k, j); per-j tensor_scalar avoids an
        # innermost stride-0 broadcast (which hangs the DVE on HW)
        qt = pool_mid.tile([PT, 2 * K * 4], F32, tag="qt")
        qt_j = qt[:].rearrange("p (a j) -> p j a", j=4)    # [p, j, (r k)]
        for j in range(4):
            ts(qt_j[:, j, :], xbso[:], float(PAMV[j]), None, OP.subtract)
        q2 = pool_mid.tile([PT, 2 * K * 4], F32, tag="q2")
        act(q2[:], qt[:], ACT.Square)

        # per-axis extremum select by sign of wr
        ex = pool_small.tile([PT, 4 * K], F32, tag="ex")    # min(2K) | max(2K)
        red(ex[:, 0:2 * K], q2[:].rearrange("p (a j) -> p a j", j=4), AX.X, OP.min)
        red(ex[:, 2 * K:4 * K], q2[:].rearrange("p (a j) -> p a j", j=4), AX.X, OP.max)
        msk = pool_small.tile([PT, 2 * K], F32, tag="msk")
        ts(ri(msk[:]), wr.unsqueeze(1).to_broadcast((PT, 2, K)), 0.0, None, OP.is_lt)
        qx = pool_small.tile([PT, 2 * K], F32, tag="qx")
        tt(qx[:], ex[:, 0:2 * K], ex[:, 2 * K:4 * K], OP.subtract)
        tt(qx[:], qx[:], msk[:], OP.mult)
        tt(qx[:], qx[:], ex[:, 2 * K:4 * K], OP.add)

        # qc = q2 - qx ; dd = qcA(rc) + qcB(ic) ; pr/pi = w * dd
        # All iteration orders chosen so no operand has an innermost stride-0
        # dim (HW DVE hazard): qc iterates [p, j, r, k]; dd/prt/pit iterate
        # [p, rc, ic, k] over flat (k, rc, ic) storage.
        qc = pool_mid.tile([PT, 2 * K * 4], F32, tag="qc")
        jrk = lambda t: t[:].rearrange("p (r k j) -> p j r k", r=2, j=4)
        qx_v = ri(qx[:]).unsqueeze(1).to_broadcast((PT, 4, 2, K))
        tt(jrk(qc), jrk(q2), qx_v, OP.subtract)

        dd = pool_big.tile([PT, K * 16], F32, tag="dd")
        prt = pool_big.tile([PT, K * 16], F32, tag="prt")
        pit = pool_big.tile([PT, K * 16], F32, tag="pit")
        abk = lambda t: t[:].rearrange("p (k a b) -> p a b k", a=4, b=4)
        qcA = qc[:, 0:K * 4].rearrange("p (k a) -> p a k", a=4).unsqueeze(2).to_broadcast((PT, 4, 4, K))
        qcB = qc[:, K * 4:2 * K * 4].rearrange("p (k b) -> p b k", b=4).unsqueeze(1).to_broadcast((PT, 4, 4, K))
        tt(abk(dd), qcA, qcB, OP.add)
        w_v = lambda a: a.unsqueeze(1).unsqueeze(1).to_broadcast((PT, 4, 4, K))
        tt(abk(prt), abk(dd), w_v(wr), OP.mult)
        tt(abk(pit), abk(dd), w_v(wi), OP.mult)

        # f = exp(pr) * (cos(pi) + i sin(pi)); ACT Sin needs args in [-pi, pi].
        # n = round(x/2pi) via the magic-number trick ((z + 1.5*2^23) - 1.5*2^23
        # rounds z to nearest integer in fp32, identical on HW and sim), then
        # Cody-Waite 2pi = C1 + C2 (C1 5-bit so n*C1 is exact, |n| <= 64):
        # r = (x - n*C1) - n*C2, accurate to ~3e-7 rad, clamped against slop.
        INV_2PI = float(1.0 / (2.0 * math.pi))
        CW1 = 6.28125
        CW2 = float(np.float32(2.0 * math.pi - 6.28125))
        PCLAMP = 3.1415925
        MAGIC = 12582912.0
        et = pool_big.tile([PT, K * 16], F32, tag="et")
        cot = pool_big.tile([PT, K * 16], F32, tag="cot")
        sit = pool_big.tile([PT, K * 16], F32, tag="sit")
        ar1 = pool_big.tile([PT, K * 16], F32, tag="ar1")
        arf = pool_big.tile([PT, K * 16], F32, tag="arf")
        act(et[:], prt[:], ACT.Exp)
        # sin(x): n = round(x/2pi)
        ts(ar1[:], pit[:], INV_2PI, MAGIC, OP.mult, OP.add)
        ts(arf[:], ar1[:], MAGIC, None, OP.subtract)
        stt(ar1[:], arf[:], -CW1, pit[:], OP.mult, OP.add)
        stt(ar1[:], arf[:], -CW2, ar1[:], OP.mult, OP.add)
        ts(ar1[:], ar1[:], -PCLAMP, PCLAMP, OP.max, OP.min)
        act(sit[:], ar1[:], ACT.Sin)
        # cos(x) = sin(x + pi/2): n = round(x/2pi + 0.25)
        ts(ar1[:], pit[:], INV_2PI, 0.25, OP.mult, OP.add)
        ts(ar1[:], ar1[:], MAGIC, MAGIC, OP.add, OP.subtract)
        stt(arf[:], ar1[:], -CW1, pit[:], OP.mult, OP.add)
        stt(arf[:], ar1[:], -CW2, arf[:], OP.mult, OP.add)
        ts(arf[:], arf[:], float(0.5 * math.pi), -PCLAMP, OP.add, OP.max)
        ts(arf[:], arf[:], PCLAMP, None, OP.min)
        act(cot[:], arf[:], ACT.Sin)
        fre = pool_big.tile([PT, K * 16], F32, tag="fre")
        fim = pool_big.tile([PT, K * 16], F32, tag="fim")
        tt(fre[:], et[:], cot[:], OP.mult)
        tt(fim[:], et[:], sit[:], OP.mult)

        # two-stage weighted sums over the 4x4 grid
        P4 = pool_mid.tile([PT, 2 * K * 4], F32, tag="P4")  # sum_ic f  (ri,k,rc)
        M4 = pool_mid.tile([PT, 2 * K * 4], F32, tag="M4")  # sum_rc f  (ri,k,ic)
        red(P4[:, 0:K * 4], fre[:].rearrange("p (a b) -> p a b", b=4), AX.X, OP.add)
        red(P4[:, K * 4:2 * K * 4], fim[:].rearrange("p (a b) -> p a b", b=4), AX.X, OP.add)
        red(M4[:, 0:K * 4].rearrange("p (k b) -> p k b", b=4),
            fre[:].rearrange("p (k a b) -> p k b a", a=4, b=4), AX.X, OP.add)
        red(M4[:, K * 4:2 * K * 4].rearrange("p (k b) -> p k b", b=4),
            fim[:].rearrange("p (k a b) -> p k b a", a=4, b=4), AX.X, OP.add)

        # SUMS [p, 8K]: Sr Si | Ur Vr Er | Ui Vi Ei
        SUMS = pool_mid.tile([PT, 8 * K], F32, tag="SUMS")

        def p3(a):
            return a.rearrange("p (x j) -> p x j", j=4)

        def rkj(t):
            return t[:].rearrange("p (r k j) -> p r k j", r=2, j=4)

        uve = SUMS[:, 2 * K:8 * K].rearrange("p (r q k) -> p r q k", r=2, q=3)
        red(SUMS[:, 0:K], p3(P4[:, 0:K * 4]), AX.X, OP.add)
        red(SUMS[:, K:2 * K], p3(P4[:, K * 4:2 * K * 4]), AX.X, OP.add)
        pj_b = pam_c.unsqueeze(1).to_broadcast((PT, 2 * K, 4))
        sj_b = psq_c.unsqueeze(1).to_broadcast((PT, 2 * K, 4))
        wk = pool_mid.tile([PT, 2 * K * 4], F32, tag="wk")
        tt(p3(wk[:]), p3(P4[:]), pj_b, OP.mult)
        red(uve[:, :, 0, :], rkj(wk), AX.X, OP.add)           # U
        tt(p3(wk[:]), p3(M4[:]), pj_b, OP.mult)
        red(uve[:, :, 1, :], rkj(wk), AX.X, OP.add)           # V
        et2 = pool_small.tile([PT, 2 * K], F32, tag="et2")
        tt(p3(wk[:]), p3(P4[:]), sj_b, OP.mult)
        red(ri(et2[:]), rkj(wk), AX.X, OP.add)
        tt(p3(wk[:]), p3(M4[:]), sj_b, OP.mult)
        red(uve[:, :, 2, :], rkj(wk), AX.X, OP.add)           # E part 2
        tt(uve[:, :, 2, :], uve[:, :, 2, :], ri(et2[:]), OP.add)

        # G block: W = 1/S ; (A,B,EW) = (U,V,E)*W
        WW = pool_small.tile([PT, 2 * K], F32, tag="WW")
        g1 = pool_small.tile([PT, 2 * K], F32, tag="g1")
        tt(g1[:, 0:K], SUMS[:, 0:K], SUMS[:, 0:K], OP.mult)
        tt(g1[:, K:2 * K], SUMS[:, K:2 * K], SUMS[:, K:2 * K], OP.mult)
        tt(g1[:, 0:K], g1[:, 0:K], g1[:, K:2 * K], OP.add)
        nc.vector.reciprocal(g1[:, K:2 * K], g1[:, 0:K])
        tt(WW[:, 0:K], SUMS[:, 0:K], g1[:, K:2 * K], OP.mult)
        stt(WW[:, K:2 * K], SUMS[:, K:2 * K], -1.0, g1[:, K:2 * K], OP.mult, OP.mult)

        def t3(a):
            return a.rearrange("p (q k) -> p q k", q=3)

        def q3b(a):
            return a.unsqueeze(1).to_broadcast((PT, 3, K))

        ABE = pool_mid.tile([PT, 6 * K], F32, tag="ABE")    # Ar Br EWr | Ai Bi EWi
        p1 = pool_mid.tile([PT, 6 * K], F32, tag="p1t")
        tt(t3(p1[:, 0:3 * K]), t3(SUMS[:, 2 * K:5 * K]), q3b(WW[:, 0:K]), OP.mult)
        tt(t3(p1[:, 3 * K:6 * K]), t3(SUMS[:, 5 * K:8 * K]), q3b(WW[:, K:2 * K]), OP.mult)
        tt(t3(ABE[:, 0:3 * K]), t3(p1[:, 0:3 * K]), t3(p1[:, 3 * K:6 * K]), OP.subtract)
        tt(t3(p1[:, 0:3 * K]), t3(SUMS[:, 2 * K:5 * K]), q3b(WW[:, K:2 * K]), OP.mult)
        tt(t3(p1[:, 3 * K:6 * K]), t3(SUMS[:, 5 * K:8 * K]), q3b(WW[:, 0:K]), OP.mult)
        tt(t3(ABE[:, 3 * K:6 * K]), t3(p1[:, 0:3 * K]), t3(p1[:, 3 * K:6 * K]), OP.add)
        Ar, Br, EWr = ABE[:, 0:K], ABE[:, K:2 * K], ABE[:, 2 * K:3 * K]
        Ai, Bi, EWi = ABE[:, 3 * K:4 * K], ABE[:, 4 * K:5 * K], ABE[:, 5 * K:6 * K]

        # x_bse
        XB = pool_small.tile([PT, 2 * K], F32, tag=f"XB{itr}")
        tt(XB[:, 0:K], Ar, Bi, OP.subtract)
        tt(XB[:, K:2 * K], Ai, Br, OP.add)

        # v_bse = EW - 2*(xr*A + xi*B) + |x|^2
        vb_new = pool_small.tile([PT, 2 * K], F32, tag=f"VB{itr}")
        tdd = pool_small.tile([PT, 2 * K], F32, tag="tdd")
        xx = pool_small.tile([PT, 2 * K], F32, tag="xx")
        tt(tdd[:], ABE[:, 0:2 * K], XB[:], OP.mult)           # xr*Ar | xi*Br
        tt(tdd[:, 0:K], tdd[:, 0:K], tdd[:, K:2 * K], OP.add)
        tt(xx[:], XB[:], XB[:], OP.mult)
        tt(xx[:, 0:K], xx[:, 0:K], xx[:, K:2 * K], OP.add)    # |x|^2
        stt(vb_new[:, 0:K], tdd[:, 0:K], -2.0, EWr, OP.mult, OP.add)
        tt(vb_new[:, 0:K], vb_new[:, 0:K], xx[:, 0:K], OP.add)
        tt(tdd[:], ABE[:, 3 * K:5 * K], XB[:], OP.mult)       # xr*Ai | xi*Bi
        tt(tdd[:, 0:K], tdd[:, 0:K], tdd[:, K:2 * K], OP.add)
        stt(vb_new[:, K:2 * K], tdd[:, 0:K], -2.0, EWi, OP.mult, OP.add)

        # g = Off @ x_bse ; u_next = bt - g
        xrB = XB[:, 0:K].unsqueeze(1).to_broadcast((PT, K, K))
        xiB = XB[:, K:2 * K].unsqueeze(1).to_broadcast((PT, K, K))
        Or3 = g3(HH[:, 0:K * K])
        Oi3 = g3(HH[:, K * K:2 * K * K])
        mv1 = pool_mid.tile([PT, 2 * K * K], F32, tag="mv1")
        mva = mv1[:, 0:K * K].rearrange("p (a b) -> p a b", a=K)
        mvb = mv1[:, K * K:2 * K * K].rearrange("p (a b) -> p a b", a=K)
        mxy = mv1[:].rearrange("p (q a b) -> p a q b", q=2, b=K)
        gt = pool_small.tile([PT, 2 * K], F32, tag="gt")
        tt(mva, Or3, xrB, OP.mult)
        stt(mvb, Oi3, -1.0, xiB, OP.mult, OP.mult)
        red(gt[:, 0:K], mxy, AX.XY, OP.add)
        tt(mva, Or3, xiB, OP.mult)
        tt(mvb, Oi3, xrB, OP.mult)
        red(gt[:, K:2 * K], mxy, AX.XY, OP.add)
        un = pool_small.tile([PT, 2 * K], F32, tag=f"un{itr}")
        tt(un[:], bt[:], gt[:], OP.subtract)

        if itr == T - 1:
            OUT = pool_out.tile([PT, 6 * K], F32, tag="OUT")
            nc.scalar.copy(OUT[:, 0:2 * K].rearrange("p (k r) -> p r k", r=2), ri(XB[:]))
            nc.scalar.copy(OUT[:, 2 * K:4 * K].rearrange("p (k r) -> p r k", r=2), ri(vb_new[:]))
            # ise = (u_next*invd - x_bse)^2 (elementwise complex square)
            tse = pool_small.tile([PT, 2 * K], F32, tag="tse")
            tt(ri(tse[:]), ri(un[:]), invd_b, OP.mult)
            tt(tse[:], tse[:], XB[:], OP.subtract)
            sqt = pool_small.tile([PT, 2 * K], F32, tag="sqt")
            tt(sqt[:], tse[:], tse[:], OP.mult)
            tt(OUT[:, 4 * K:6 * K:2], sqt[:, 0:K], sqt[:, K:2 * K], OP.subtract)
            stt(OUT[:, 4 * K + 1:6 * K:2], tse[:, 0:K], 2.0, tse[:, K:2 * K], OP.mult, OP.mult)
            nc.gpsimd.dma_start(out_d[bass.ds(row, PT), :], OUT[:])
        else:
            u_t = un
            vb_prev = vb_new


def build_program(T, np_core=NP, unroll=UNROLL):
    """Trace the single-core program; returns nc."""
    nt = np_core // PT
    nc = bacc.Bacc(trn_type="TRN2")
    hin = nc.dram_tensor("hin", [np_core, 2 * M * K], F32, kind="ExternalInput")
    yin = nc.dram_tensor("yin", [np_core, 2 * M], F32, kind="ExternalInput")
    cin = nc.dram_tensor("cin", [PT, CW], F32, kind="ExternalInput")
    outp = nc.dram_tensor("outp", [np_core, 6 * K], F32, kind="ExternalOutput")

    with tile.TileContext(nc) as tc:
        with (
            tc.tile_pool(name="pin", bufs=2) as pool_in,
            tc.tile_pool(name="pbig", bufs=2) as pool_big,
            tc.tile_pool(name="pmid", bufs=2) as pool_mid,
            tc.tile_pool(name="psmall", bufs=2) as pool_small,
            tc.tile_pool(name="pout", bufs=2) as pool_out,
            tc.tile_pool(name="pconst", bufs=1) as pool_const,
        ):
            cin_t = pool_const.tile([PT, CW], F32, tag="cin")
            nc.gpsimd.dma_start(cin_t[:], cin[:, :])
            pools = (pool_in, pool_big, pool_mid, pool_small, pool_out)
            hin_a, yin_a, out_a = hin[:, :], yin[:, :], outp[:, :]
            if nt > unroll:
                assert nt % unroll == 0
                with tc.For_i(0, nt // unroll, 1) as iv:
                    for j in range(unroll):
                        _emit_ptile(nc, pools, hin_a, yin_a, out_a, cin_t[:],
                                    iv * (unroll * PT) + j * PT, T)
            else:
                for j in range(nt):
                    _emit_ptile(nc, pools, hin_a, yin_a, out_a, cin_t[:], j * PT, T)
    nc.compile()
    return nc


_PROGRAMS = {}


def _get_program(T):
    if T not in _PROGRAMS:
        _PROGRAMS[T] = build_program(T)
    return _PROGRAMS[T]


def _make_consts(no_val):
    c = np.zeros((PT, CW), dtype=np.float32)
    c[:, C_PAM:C_PAM + 4] = PAMV
    c[:, C_PSQ:C_PSQ + 4] = PAMV * PAMV
    c[:, C_X0R] = np.float32(X0.real)
    c[:, C_X0I] = np.float32(X0.imag)
    c[:, C_NX0I] = -np.float32(X0.imag)
    c[:, C_V0] = V0
    c[:, C_C0] = np.float32(no_val) * np.float32(no_val)
    c[:, C_HPI] = np.float32(math.pi / 2.0)
    c[:, C_NPI] = -np.float32(math.pi)
    return c


def kernel(y, h, prior, no, it, v_dsc_prev, x_bse_prev, v_bse_prev,
           ise_dsc_prev, test, x_dsc_prev, internal_it):
    y = np.ascontiguousarray(np.asarray(y, dtype=np.complex64)).reshape(B * S, M)
    h = np.ascontiguousarray(np.asarray(h, dtype=np.complex64)).reshape(B * S, M * K)
    no_val = float(np.asarray(no, dtype=np.float32).reshape(-1)[-1])
    T = int(internal_it)
    nc = _get_program(T)
    consts = _make_consts(no_val)
    in_maps = []
    for cidx in range(NCORES):
        sl = slice(cidx * NP, (cidx + 1) * NP)
        in_maps.append({
            "hin": np.ascontiguousarray(h[sl]).view(np.float32),
            "yin": np.ascontiguousarray(y[sl]).view(np.float32),
            "cin": consts,
        })
    res = bass_utils.run_bass_kernel_spmd(nc, in_maps, core_ids=list(range(NCORES)))
    out = np.concatenate([r["outp"] for r in res.results], axis=0)  # [B*S, 48] f32
    outc = out.view(np.complex64)                                   # [B*S, 24]
    sh = (B, 1, 1, S, K, 1)
    x_bse = np.ascontiguousarray(outc[:, 0:K]).reshape(sh)
    v_bse = np.ascontiguousarray(outc[:, K:2 * K]).reshape(sh)
    ise = np.ascontiguousarray(outc[:, 2 * K:3 * K]).reshape(sh)
    return x_bse, v_bse, ise
